# revision 26
# baseline (speedup 1.0000x reference)
"""Trainium2 Bass kernel for nn_Model_15418932592810 (Autoformer-style decoder layer).

Sharding: data-parallel over batch B=8 across the 8 NeuronCores (one batch
per core, no collectives). Within a core, activations are feature-major
through the matmul chains; attention is keys-major (scores^T) with softmax
denominators riding as a ones-column in V; the series-decomp / layernorm
tail runs token-major.

v2 notes:
- all attention operands bf16 (K<128 fp32r matmuls run at ~half rate on HW)
- softmax/LN reciprocals via exp(-ln x) on the activation engine, staying
  inside one act-function table (exp/ln/relu/square/copy) -- no DVE
  RECIPROCAL (8.7us per call on a 1-partition row), no table reloads
- sa decay bias applied as a post-exp multiply (exp(s+b)=exp(s)exp(b));
  when all heads share lambda (true for these inputs) one packed tile
  serves all heads
- sa out-proj folded into fusion W1, cr out-proj folded into the ctx-pool
  K/V projections (host-side weight products); ctx-pool K bias dropped
  (adds a per-head constant to logits -- softmax invariant)
- weights DMA'd in a few large chunks, issued in consumption order
- trend's (A@z)^T produced directly by matmul (z as lhsT) instead of PE
  transposes
- PE warm-up spins while the first weight chunks land (HAM stays at 8/8)
"""
import math
import os
import numpy as np
import ml_dtypes

import concourse.bass as bass
import concourse.mybir as mybir
import concourse.tile as tile
from concourse import bacc
from concourse.bass_utils import run_bass_kernel_spmd

F32 = mybir.dt.float32
F32R = mybir.dt.float32r
BF16 = mybir.dt.bfloat16
AX = mybir.AxisListType
ALU = mybir.AluOpType
ACTF = mybir.ActivationFunctionType

B, L, D, H, DH, DFF, KMA = 8, 512, 512, 8, 64, 2048, 25
NT = 4          # number of 128-row tiles in a 512 dim
NF = DFF // 128
EPS = 1e-5
BF = ml_dtypes.bfloat16

# row indices in the packed (NROWS, 512) f32 "rows" tensor
R_SA_BV, R_CF_BV, R_CR_BV, R_MHA_BV, R_MHA_BO, R_FUS_B, R_TREND_B = range(7)
NROWS = 7
COLS_W = 192      # 24 bias cols + pad + identity(128) at the tail


def r(x):
    return x.bitcast(F32R)


def mktile(pool, shape, dtype, tag, bufs=None):
    return pool.tile(shape, dtype, name=tag, tag=tag, bufs=bufs)


# ----------------------------------------------------------------------------
# host-side input preparation
# ----------------------------------------------------------------------------

def _softplus(x):
    return np.logaddexp(0.0, x.astype(np.float64))


def _ma_matrix():
    pad = (KMA - 1) // 2
    A = np.zeros((L, L), dtype=np.float64)
    for i in range(L):
        for m in range(i, i + KMA):
            j = min(max(m - pad, 0), L - 1)
            A[i, j] += 1.0 / KMA
    return A


def _colpack(x):
    n = np.asarray(x).shape[0]
    return np.asarray(x, np.float64).reshape(n // 128, 128).T


def _T(w):
    return np.asarray(w, dtype=np.float64).T


def _pack(a):
    # (R, N) with R=128*c -> (128, c*N): column block c holds rows [128c,128c+128)
    a = np.asarray(a)
    rr, n = a.shape
    c = rr // 128
    return np.ascontiguousarray(
        a.reshape(c, 128, n).transpose(1, 0, 2).reshape(128, c * n))


def _Tp(w):
    return _pack(_T(w))


def host_prepare(inputs):
    ins = {k: np.asarray(v, dtype=np.float64) for k, v in inputs.items()}
    sh = {}
    s = 1.0 / math.sqrt(DH)

    qkv_w = ins["sa_qkv_w"]
    qkv_b = ins["sa_qkv_b"]
    att = {
        "sa": (qkv_w[:D] * s, qkv_b[:D] * s, qkv_w[D:2 * D], qkv_b[D:2 * D],
               qkv_w[2 * D:], qkv_b[2 * D:]),
        "cf": (ins["cf_q_w"] * s, ins["cf_q_b"] * s, ins["cf_k_w"],
               ins["cf_k_b"], ins["cf_v_w"], ins["cf_v_b"]),
        "cr": (ins["cr_q_w"] * s, ins["cr_q_b"] * s, ins["cr_k_w"],
               ins["cr_k_b"], ins["cr_v_w"], ins["cr_v_b"]),
    }

    def bfc(*mats):
        return np.ascontiguousarray(
            np.concatenate([_Tp(m) for m in mats], axis=1)).astype(BF)

    i = np.arange(L, dtype=np.float64)
    rel = i[None, :] - i[:, None]                  # rel[q, k] = k - q
    lf = _softplus(ins["sa_lam_f"])[:, None, None]
    lb = _softplus(ins["sa_lam_b"])[:, None, None]
    decay = np.where(rel[None] < 0, -lb * np.abs(rel[None]),
                     np.where(rel[None] > 0, -lf * rel[None], 0.0))
    edecay = np.exp(decay)                          # (H, L, L), in [0, 1]
    share_bias = bool(np.all(np.abs(edecay - edecay[:1]) < 1e-12))

    wq, bq, wk, bk, wv, bv = att["sa"]
    if share_bias:
        sa_chunk = np.concatenate(
            [bfc(wq, wk, wv), _pack(edecay[0].T).astype(BF)], axis=1)
    else:
        sa_chunk = np.concatenate(
            [bfc(wq, wk, wv)] + [_pack(m.T).astype(BF) for m in edecay],
            axis=1)
    sh["sa_w"] = np.ascontiguousarray(sa_chunk)

    sh["cf_w"] = bfc(att["cf"][0], att["cf"][2], att["cf"][4], ins["cf_o_w"])
    sh["cr_w"] = bfc(att["cr"][0], att["cr"][2], att["cr"][4])

    for p, lw in [("cf", "cf_logw"), ("cr", "cr_logw")]:
        w = np.exp(ins[lw])[:, None]
        ang = 2.0 * math.pi * w * i[None, :]       # (H, L)
        cs = np.stack([np.cos(ang), np.sin(ang)], axis=1)  # (H, 2, L)
        cs2 = cs.transpose(1, 0, 2).reshape(2, H * L)
        # rows 0-1 serve even heads, rows 64-65 odd heads, so consecutive
        # cs-init matmuls alternate PE row-groups (LDWEIGHTS overlap)
        cst = np.zeros((66, H * L))
        cst[0:2] = cs2
        cst[64:66] = cs2
        sh[f"{p}_cs"] = np.ascontiguousarray(cst).astype(BF)

    # folded weights
    wqm = ins["mha_in_w"][:D]
    bqm = ins["mha_in_b"][:D]
    wkm = ins["mha_in_w"][D:2 * D]
    wvm = ins["mha_in_w"][2 * D:]
    bvm = ins["mha_in_b"][2 * D:]
    wo_cr, bo_cr = ins["cr_o_w"], ins["cr_o_b"]
    wk_f = wkm @ wo_cr
    wv_f = wvm @ wo_cr
    bv_f = bvm + wvm @ bo_cr
    w1 = ins["fusion_w"][:, :D]
    wc = w1 @ ins["sa_out_w"]
    fus_b = ins["fusion_b"] + w1 @ ins["sa_out_b"]
    sh["mix_w"] = bfc(wk_f, wv_f, ins["mha_out_w"], wc, ins["fusion_w"][:, D:])

    sh["conv_w"] = np.ascontiguousarray(np.concatenate(
        [_Tp(ins["conv1_w"] * ins["norm3_g"][None, :]),
         _Tp(ins["conv2_w"])], axis=1)).astype(BF)

    sh["trend_wT"] = _Tp(ins["trend_w"]).astype(BF)
    sh["A_lhsT"] = _Tp(_ma_matrix()).astype(np.float32)
    sh["gf_rep"] = np.ascontiguousarray(
        np.tile(np.asarray(ins["normf_g"], np.float32)[None, :], (128, 1)))

    cols = np.zeros((128, COLS_W), np.float64)
    for idx, pfx in enumerate(("sa", "cf", "cr")):
        cols[:, 8 * idx:8 * idx + 4] = _colpack(att[pfx][1])
        cols[:, 8 * idx + 4:8 * idx + 8] = _colpack(att[pfx][3])
    cols[:, 24:28] = _colpack(ins["cf_o_b"])
    cols[:, COLS_W - 128:] = np.eye(128)
    sh["cols"] = cols.astype(np.float32)

    rows = np.zeros((NROWS, 512), np.float64)
    rows[R_SA_BV] = att["sa"][5]
    rows[R_CF_BV] = att["cf"][5]
    rows[R_CR_BV] = att["cr"][5]
    rows[R_MHA_BV] = bv_f
    rows[R_MHA_BO] = ins["mha_out_b"]
    rows[R_FUS_B] = fus_b
    rows[R_TREND_B] = ins["trend_b"]
    sh["rows"] = rows.astype(np.float32)

    qvec = (ins["global_q"].reshape(D) @ wqm.T + bqm) * s
    qvp = np.zeros((128, 32), np.float64)
    for h in range(H):
        fc = (64 * h) // 128
        r0 = 64 * h - 128 * fc
        qvp[r0:r0 + 64, 8 * fc + h] = qvec[64 * h:64 * h + 64]
    sh["qv_bf"] = qvp.astype(BF)

    per_core = []
    for b in range(B):
        x_sa = np.asarray(inputs["x_sa"][b], np.float64)
        per_core.append({
            "xsaT": _Tp(x_sa).astype(BF),
            "xq": np.ascontiguousarray(np.concatenate(
                [_Tp(np.asarray(inputs["x_q1"][b], np.float64)),
                 _Tp(np.asarray(inputs["x_q2"][b], np.float64))],
                axis=1)).astype(BF),
            "xsa_tok": _pack(x_sa).astype(np.float32),
        })
    return sh, per_core, share_bias


# ----------------------------------------------------------------------------
# program builder
# ----------------------------------------------------------------------------

def _specs(share_bias):
    sa_w_cols = 3 * 2048 + (2048 if share_bias else 8 * 2048)
    shared = [
        ("cols", (128, COLS_W), F32), ("rows", (NROWS, 512), F32),
        ("qv_bf", (128, 32), BF16),
        ("sa_w", (128, sa_w_cols), BF16),
        ("cf_w", (128, 4 * 2048), BF16), ("cf_cs", (66, H * L), BF16),
        ("cr_w", (128, 3 * 2048), BF16), ("cr_cs", (66, H * L), BF16),
        ("mix_w", (128, 5 * 2048), BF16),
        ("conv_w", (128, 16384), BF16),
        ("trend_wT", (128, 2048), BF16),
        ("A_lhsT", (128, 2048), F32),
        ("gf_rep", (128, 512), F32),
    ]
    per_core = [
        ("xsaT", (128, 2048), BF16), ("xq", (128, 4096), BF16),
        ("xsa_tok", (128, 2048), F32),
    ]
    outs = [("out_x", (512, 512), F32), ("out_trend", (512, 512), F32)]
    return shared, per_core, outs


def hslice(tiles, h):
    t = tiles[h // 2]
    off = 64 * (h % 2)
    return t[off:off + 64, :]


class Prog:
    def __init__(self, nc, tc, Hd):
        self.nc, self.tc, self.Hd = nc, tc, Hd

    # ------------------------------------------------------------------
    def chunk(self, pool, name, width, dtype=BF16, tag=None):
        nc = self.nc
        t = mktile(pool, [128, width], dtype, tag or name)
        if dtype == F32:
            nc.sync.dma_start(r(t[:]), r(self.Hd[name][:]))
        else:
            nc.sync.dma_start(t[:], self.Hd[name][:])
        return t

    def rowtile(self, pool, ridx, tag):
        """(1, 512) f32 tile at partition 0, loaded from rows[ridx]."""
        t = mktile(pool, [1, 512], F32, tag)
        self.nc.sync.dma_start(r(t[:]), r(self.Hd["rows"][ridx:ridx + 1, :]))
        return t

    @staticmethod
    def views(t, n, width=512, off=0):
        return [t[:, off + width * c:off + width * (c + 1)] for c in range(n)]

    def ps_mm(self):
        return mktile(self.psp, [128, 512], F32, "mm", bufs=2)

    def ps_sc(self):
        return mktile(self.psp, [128, 512], F32, "sc", bufs=2)

    def ps_sc2(self):
        return mktile(self.psp, [128, 1024], F32, "sc", bufs=2)

    def ps_av(self):
        return mktile(self.psp, [65, 512], F32, "av", bufs=2)

    # ------------------------------------------------------------------
    def proj_fm(self, xT, wT, b_c, tag, dtype=BF16, bufs=5):
        """Feature-major projection: out^T = W @ x^T + b, NT tiles (128,512)."""
        nc = self.nc
        outs = []
        for m in range(NT):
            ps = self.ps_mm()
            for kc in range(NT):
                nc.tensor.matmul(ps[:], wT[kc][:, 128 * m:128 * (m + 1)],
                                 xT[kc][:], start=(kc == 0),
                                 stop=(kc == NT - 1))
            o = mktile(self.sbp, [128, 512], dtype, tag, bufs=bufs)
            nc.vector.tensor_scalar(o[:], ps[:], b_c[:, m:m + 1], None,
                                    op0=ALU.add)
            outs.append(o)
        return outs

    def proj_tok_aug(self, xT, wvT, bv_row, tag):
        """Token-major V projection, ones column interleaved per head (bf16)."""
        nc = self.nc
        bv_rep = mktile(self.sbp, [128, 512], F32, "at_bvrep", bufs=1)
        nc.gpsimd.partition_broadcast(bv_rep[:], bv_row[:])
        outs = []
        for kc in range(NT):
            ps = self.ps_mm()
            for fc in range(NT):
                nc.tensor.matmul(ps[:], xT[fc][:, 128 * kc:128 * (kc + 1)],
                                 wvT[fc][:], start=(fc == 0),
                                 stop=(fc == NT - 1))
            o = mktile(self.sbp, [128, 520], BF16, tag, bufs=5)
            nc.vector.memset(o[:], 1.0)
            ov = o[:].rearrange("p (h c) -> p h c", c=65)
            nc.vector.tensor_tensor(
                ov[:, :, 0:64], ps[:].rearrange("p (h c) -> p h c", c=64),
                bv_rep[:].rearrange("p (h c) -> p h c", c=64), op=ALU.add)
            outs.append(o)
        return outs

    # ------------------------------------------------------------------
    def attention(self, qinT, kvinT, wqT, wkT, wvT, bq_c, bk_c, bv_row,
                  decay=None, cs=None, out_proj=None, aout_pool=None,
                  aout_tag="at_ao"):
        """8-head attention; returns feature-major bf16 NT tiles: the
        pre-out-proj attention concat, or (with out_proj=(woT, bo_c, residT))
        the projected + residual output."""
        nc = self.nc
        qT = self.proj_fm(qinT, wqT, bq_c, "at_q")
        kT = self.proj_fm(kvinT, wkT, bk_c, "at_k")
        vaug = self.proj_tok_aug(kvinT, wvT, bv_row, "at_v")
        apool = aout_pool or self.sbp

        aout = []
        for half in range(2):
            expT = [mktile(self.sbp, [128, 2048], BF16, "at_exp", bufs=5)
                    for _ in range(NT)]
            avs_sb = []
            zcat = mktile(self.sbp, [1, 2048], F32, "at_zc", bufs=1)
            for pair in range(2):
                ha, hb = 4 * half + 2 * pair, 4 * half + 2 * pair + 1
                for kc in range(NT):
                    # two heads per 2-bank psum tile; emission order
                    # cs-a, cs-b, score-a, score-b alternates PE row-groups
                    # on every instruction so LDWEIGHTS overlaps matmuls
                    ps = self.ps_sc2()
                    for j, h in enumerate((ha, hb)):
                        out = ps[:, 512 * j:512 * (j + 1)]
                        if cs is not None:
                            cr0 = 0 if h % 2 == 0 else 64
                            nc.tensor.matmul(
                                out,
                                cs[cr0:cr0 + 2,
                                   512 * h + 128 * kc:512 * h + 128 * (kc + 1)],
                                cs[cr0:cr0 + 2, 512 * h:512 * (h + 1)],
                                start=True, stop=False)
                        nc.tensor.matmul(
                            out,
                            hslice(kT, h)[:, 128 * kc:128 * (kc + 1)],
                            hslice(qT, h), start=(cs is None), stop=True)
                    esl = expT[kc][:, 1024 * pair:1024 * (pair + 1)]
                    nc.scalar.activation(esl, ps[:], ACTF.Exp)
                    if decay is not None:
                        dsl = decay[ha] if isinstance(decay, list) else decay
                        db = dsl[:, 512 * kc:512 * (kc + 1)]
                        nc.vector.tensor_tensor(
                            esl[:, 0:512], esl[:, 0:512], db, op=ALU.mult)
                        if isinstance(decay, list):
                            db = decay[hb][:, 512 * kc:512 * (kc + 1)]
                        nc.vector.tensor_tensor(
                            esl[:, 512:1024], esl[:, 512:1024], db,
                            op=ALU.mult)
                for j, h in enumerate((ha, hb)):
                    av = self.ps_av()
                    for kc in range(NT):
                        nc.tensor.matmul(
                            av[:], vaug[kc][:, 65 * h:65 * h + 65],
                            expT[kc][:, 1024 * pair + 512 * j:
                                     1024 * pair + 512 * (j + 1)],
                            start=(kc == 0), stop=(kc == NT - 1))
                    h4 = 2 * pair + j
                    nc.vector.tensor_copy(
                        zcat[0:1, 512 * h4:512 * (h4 + 1)], av[64:65, :])
                    asb = mktile(self.sbp, [64, 512], BF16, "at_asb", bufs=6)
                    nc.vector.tensor_copy(asb[:], av[0:64, :])
                    avs_sb.append(asb)
            # softmax epilogue with a single Ln per half: gather the four
            # Z-rows into one (1,2048) row, one Ln, one wide exp(-x) -- the
            # rcp shares the exp table with the score exps, so each half
            # costs at most 2 act-table loads
            nc.scalar.activation(zcat[:], zcat[:], ACTF.Ln)
            rcp = mktile(self.sbp, [1, 2048], BF16, "at_rcp", bufs=2)
            nc.scalar.activation(rcp[:], zcat[:], ACTF.Exp, scale=-1.0)
            for h4 in range(4):
                h = 4 * half + h4
                rep = mktile(self.sbp, [64, 512], BF16, "at_rep", bufs=4)
                nc.gpsimd.partition_broadcast(
                    rep[:], rcp[0:1, 512 * h4:512 * (h4 + 1)])
                if h % 2 == 0:
                    pair = mktile(apool, [128, 512], BF16, aout_tag, bufs=4)
                    aout.append(pair)
                off = 64 * (h % 2)
                nc.vector.tensor_tensor(aout[h // 2][off:off + 64, :],
                                        avs_sb[h4][:], rep[:], op=ALU.mult)
        if out_proj is None:
            return aout
        woT, bo_c, residT = out_proj
        outs = []
        for m in range(NT):
            ps = self.ps_mm()
            for c in range(NT):
                nc.tensor.matmul(ps[:], woT[c][:, 128 * m:128 * (m + 1)],
                                 aout[c][:], start=(c == 0),
                                 stop=(c == NT - 1))
            o = mktile(self.sbp, [128, 512], BF16, "at_enr", bufs=4)
            nc.vector.tensor_scalar(o[:], ps[:], bo_c[:, m:m + 1], None,
                                    op0=ALU.add)
            nc.vector.tensor_tensor(o[:], o[:], residT[m][:], op=ALU.add)
            outs.append(o)
        return outs

    # ------------------------------------------------------------------
    def ctx_pool_fusion(self, caT, wkT, wvT, woT, w2T, qv, bv_row, bo_row,
                        fus_b_row, s2b, pool):
        """1-query MultiheadAttention pooling feeding the fusion summary row:
        s2b = Wf2 @ summary + fus_b'. K-proj bias dropped (softmax-invariant).
        Tiles live in `pool` (keep) so the attn pool can close beforehand and
        the tail overlaps the ctx chain."""
        nc = self.nc
        sav_sbp, self.sbp = self.sbp, pool
        kT = self.proj_fm(caT, wkT, self.zero_col[:, 0:4], "mha_k", bufs=4)
        v_tok = []
        for kc in range(NT):
            ps = self.ps_mm()
            nc.tensor.matmul(ps[:], r(self.ones1x128[:]), r(bv_row[:]),
                             start=True, stop=False)
            for fc in range(NT):
                nc.tensor.matmul(ps[:], caT[fc][:, 128 * kc:128 * (kc + 1)],
                                 wvT[fc][:], start=False, stop=(fc == NT - 1))
            o = mktile(self.sbp, [128, 512], BF16, "mha_v", bufs=4)
            nc.any.tensor_copy(o[:], ps[:])
            v_tok.append(o)
        exps = []
        for kc in range(NT):
            ps = self.ps_sc()
            for fc in range(NT):
                nc.tensor.matmul(ps[:, 0:8],
                                 kT[fc][:, 128 * kc:128 * (kc + 1)],
                                 qv[:, 8 * fc:8 * (fc + 1)],
                                 start=(fc == 0), stop=(fc == NT - 1))
            e = mktile(self.sbp, [128, 8], BF16, "mha_exp", bufs=4)
            nc.scalar.activation(e[:], ps[:, 0:8], ACTF.Exp)
            exps.append(e)
        sps = self.ps_sc()
        for kc in range(NT):
            nc.tensor.matmul(sps[0:1, 0:8], self.ones_col_bf[:, 0:1],
                             exps[kc][:], start=(kc == 0), stop=(kc == NT - 1))
        lnz = mktile(self.sbp, [1, 8], F32, "mha_lnz")
        nc.scalar.activation(lnz[:], sps[0:1, 0:8], ACTF.Ln)
        rsum = mktile(self.sbp, [1, 8], F32, "mha_rsum")
        nc.scalar.activation(rsum[:], lnz[:], ACTF.Exp, scale=-1.0)
        rrep = mktile(self.sbp, [128, 8], F32, "mha_rrep")
        nc.gpsimd.partition_broadcast(rrep[:], rsum[:])
        for kc in range(NT):
            nc.vector.tensor_tensor(exps[kc][:], exps[kc][:], rrep[:],
                                    op=ALU.mult)

        yps = self.ps_av()
        for h in range(H):
            for kc in range(NT):
                nc.tensor.matmul(yps[0:64, 2 * h:2 * h + 1],
                                 v_tok[kc][:, 64 * h:64 * h + 64],
                                 exps[kc][:, h:h + 1],
                                 start=(kc == 0), stop=(kc == NT - 1))
        y_sb = mktile(self.sbp, [128, NT], BF16, "mha_y")
        nc.vector.memset(y_sb[:], 0.0)
        for h in range(H):
            off = 64 * (h % 2)
            nc.vector.tensor_copy(y_sb[off:off + 64, h // 2:h // 2 + 1],
                                  yps[0:64, 2 * h:2 * h + 1])

        summ = mktile(self.sbp, [128, 8], BF16, "mha_summ")
        sps2 = self.ps_mm()
        for mc in range(NT):
            nc.tensor.matmul(sps2[:, 2 * mc:2 * mc + 1],
                             bo_row[0:1, 128 * mc:128 * (mc + 1)],
                             self.ones1x1[:], start=True, stop=False)
            for c in range(NT):
                nc.tensor.matmul(sps2[:, 2 * mc:2 * mc + 1],
                                 woT[c][:, 128 * mc:128 * (mc + 1)],
                                 y_sb[:, c:c + 1], start=False,
                                 stop=(c == NT - 1))
        for mc in range(NT):
            nc.vector.tensor_copy(summ[:, 2 * mc:2 * mc + 1],
                                  sps2[:, 2 * mc:2 * mc + 1])

        ps = self.ps_sc()
        nc.tensor.matmul(ps[0:1, :], r(self.ones1x1[:]), r(fus_b_row[:]),
                         start=True, stop=False)
        for fc in range(NT):
            nc.tensor.matmul(ps[0:1, :], summ[:, 2 * fc:2 * fc + 1],
                             w2T[fc][:], start=False, stop=(fc == NT - 1))
        nc.vector.tensor_copy(r(s2b[:]), ps[0:1, :])
        self.sbp = sav_sbp

    # ------------------------------------------------------------------
    def my_ln_stats(self, y_tiles, tag):
        nc = self.nc
        sums = mktile(self.sbp, [128, NT], F32, f"{tag}_sums")
        sumsq = mktile(self.sbp, [128, NT], F32, f"{tag}_sumsq")
        for c in range(NT):
            nc.vector.tensor_reduce(sums[:, c:c + 1], y_tiles[c][:], axis=AX.X,
                                    op=ALU.add)
            sq = mktile(self.sbp, [128, 512], BF16, "lnsq", bufs=2)
            nc.scalar.activation(sq[:], y_tiles[c][:], ACTF.Square,
                                 accum_out=sumsq[:, c:c + 1])
        mu = mktile(self.sbp, [128, NT], F32, f"{tag}_mu")
        nc.vector.tensor_scalar(mu[:], sums[:], 1.0 / D, None, op0=ALU.mult)
        var = mktile(self.sbp, [128, NT], F32, f"{tag}_var")
        nc.vector.tensor_tensor(var[:], mu[:], mu[:], op=ALU.mult)
        msq = mktile(self.sbp, [128, NT], F32, f"{tag}_msq")
        nc.vector.tensor_scalar(msq[:], sumsq[:], 1.0 / D, None, op0=ALU.mult)
        nc.vector.tensor_tensor(var[:], msq[:], var[:], op=ALU.subtract)
        # rstd = exp(-0.5 * ln(var + eps)): stays in the exp/ln act table
        lnv = mktile(self.sbp, [128, NT], F32, f"{tag}_lnv")
        nc.scalar.activation(lnv[:], var[:], ACTF.Ln,
                             bias=self.eps_col[:, 0:1])
        rstd = mktile(self.sbp, [128, NT], F32, f"{tag}_rstd")
        nc.scalar.activation(rstd[:], lnv[:], ACTF.Exp, scale=-0.5)
        return mu, rstd

    def seqmean_sub(self, xh_tiles, tag, gmul=None, outs=None):
        nc = self.nc
        ps = self.ps_sc()
        for c in range(NT):
            nc.tensor.matmul(ps[0:1, :], r(self.ones_col[:, 0:1]),
                             r(xh_tiles[c][:]), start=(c == 0),
                             stop=(c == NT - 1))
        row = mktile(self.sbp, [1, 512], F32, "sm_row", bufs=1)
        nc.scalar.mul(row[:], ps[0:1, :], 1.0 / L)
        rep = mktile(self.sbp, [128, 512], F32, "sm_rep", bufs=2)
        nc.gpsimd.partition_broadcast(rep[:], row[:])
        if outs is None:
            outs = [mktile(self.sbp, [128, 512], F32, f"{tag}_out", bufs=4)
                    for _ in range(NT)]
        for c in range(NT):
            nc.vector.tensor_tensor(r(outs[c][:]), xh_tiles[c][:], rep[:],
                                    op=ALU.subtract)
            if gmul is not None:
                nc.vector.tensor_tensor(r(outs[c][:]), outs[c][:], gmul[:],
                                        op=ALU.mult)
        return outs

    def ma_matmul(self, A_lhsT, x_tiles):
        nc = self.nc
        pss = []
        for tc_ in range(NT):
            ps = self.ps_mm()
            for kc in range(NT):
                nc.tensor.matmul(ps[:],
                                 r(A_lhsT[kc][:, 128 * tc_:128 * (tc_ + 1)]),
                                 r(x_tiles[kc][:]), start=(kc == 0),
                                 stop=(kc == NT - 1))
            pss.append(ps)
        return pss


def emit(tc, nc, Hd, share_bias):
    p = Prog(nc, tc, Hd)
    keep_cm = tc.tile_pool(name="keep", bufs=1)
    keep = keep_cm.__enter__()
    psp_cm = tc.tile_pool(name="ps", bufs=1, space="PSUM")
    p.psp = psp_cm.__enter__()

    # constants: one cols DMA + memsets; per-row tiles from the rows tensor
    cols = p.chunk(keep, "cols", COLS_W, dtype=F32)
    p.identf = cols[:, COLS_W - 128:COLS_W]
    bcol = {}
    for idx, pfx in enumerate(("sa", "cf", "cr")):
        bcol[f"{pfx}_q"] = cols[:, 8 * idx:8 * idx + 4]
        bcol[f"{pfx}_k"] = cols[:, 8 * idx + 4:8 * idx + 8]
    cf_bo_c = cols[:, 24:28]
    qv = mktile(keep, [128, 32], BF16, "qv_bf")
    nc.sync.dma_start(qv[:], Hd["qv_bf"][:])
    p.zero_col = mktile(keep, [128, 4], F32, "zero_col")
    nc.vector.memset(p.zero_col[:], 0.0)
    p.ones_col = mktile(keep, [128, 1], F32, "ones_col")
    nc.vector.memset(p.ones_col[:], 1.0)
    p.ones_col_bf = mktile(keep, [128, 1], BF16, "ones_col_bf")
    nc.vector.memset(p.ones_col_bf[:], 1.0)
    p.ones1x128 = mktile(keep, [1, 128], F32, "ones1x128")
    nc.vector.memset(p.ones1x128[:], 1.0)
    p.ones1x1 = p.ones1x128[0:1, 0:1]
    p.eps_col = mktile(keep, [128, 1], F32, "eps_col")
    nc.vector.memset(p.eps_col[:], EPS)
    sa_bv = p.rowtile(keep, R_SA_BV, "sa_bv")
    cf_bv = p.rowtile(keep, R_CF_BV, "cf_bv")
    cr_bv = p.rowtile(keep, R_CR_BV, "cr_bv")
    mha_bv = p.rowtile(keep, R_MHA_BV, "mha_bv")
    mha_bo = p.rowtile(keep, R_MHA_BO, "mha_bo")
    fus_b = p.rowtile(keep, R_FUS_B, "fus_b")
    trend_b = p.rowtile(keep, R_TREND_B, "trend_b")
    s2b = mktile(keep, [1, 512], F32, "s2b_row")

    # PE warm-up spins on the identity while the first weight chunks land
    for _ in range(24):
        ps = p.ps_sc()
        nc.tensor.matmul(ps[:, 0:128], r(p.identf), r(p.identf),
                         start=True, stop=True)

    attn_cm = tc.tile_pool(name="attn", bufs=1)
    ap = attn_cm.__enter__()
    p.sbp = ap

    # weight chunks, issued in consumption order
    xsaT = p.views(p.chunk(ap, "xsaT", 2048), NT)
    sa_w_cols = 3 * 2048 + (2048 if share_bias else 8 * 2048)
    sa_w = p.chunk(ap, "sa_w", sa_w_cols)
    xq_t = p.chunk(ap, "xq", 4096)
    xq1T, xq2T = p.views(xq_t, NT), p.views(xq_t, NT, off=2048)
    cf_w = p.chunk(ap, "cf_w", 4 * 2048)
    cf_cs = mktile(ap, [66, H * L], BF16, "cf_cs")
    nc.sync.dma_start(cf_cs[:], Hd["cf_cs"][:])
    cr_w = p.chunk(ap, "cr_w", 3 * 2048)
    cr_cs = mktile(ap, [66, H * L], BF16, "cr_cs")
    nc.sync.dma_start(cr_cs[:], Hd["cr_cs"][:])
    mix_w = p.chunk(keep, "mix_w", 5 * 2048)
    xsa_tok = p.views(p.chunk(keep, "xsa_tok", 2048, dtype=F32), NT)
    gf_rep = p.chunk(keep, "gf_rep", 512, dtype=F32)

    sa_wq = p.views(sa_w, NT)
    sa_wk = p.views(sa_w, NT, off=2048)
    sa_wv = p.views(sa_w, NT, off=2 * 2048)
    if share_bias:
        sa_decay = sa_w[:, 3 * 2048:4 * 2048]
    else:
        sa_decay = [sa_w[:, (3 + h) * 2048:(4 + h) * 2048] for h in range(H)]

    aout_sa = p.attention(xsaT, xsaT, sa_wq, sa_wk, sa_wv,
                          bcol["sa_q"], bcol["sa_k"], sa_bv,
                          decay=sa_decay, aout_pool=keep, aout_tag="aout_sa")
    cfv = [p.views(cf_w, NT, off=2048 * i) for i in range(4)]
    enriched = p.attention(xq2T, xq1T, cfv[0], cfv[1], cfv[2],
                           bcol["cf_q"], bcol["cf_k"], cf_bv, cs=cf_cs,
                           out_proj=(cfv[3], cf_bo_c, xq2T))
    crv = [p.views(cr_w, NT, off=2048 * i) for i in range(3)]
    aout_cr = p.attention(xq1T, enriched, crv[0], crv[1], crv[2],
                          bcol["cr_q"], bcol["cr_k"], cr_bv, cs=cr_cs,
                          aout_pool=keep, aout_tag="aout_cr")

    mixv = [p.views(mix_w, NT, off=2048 * i) for i in range(5)]
    p.ctx_pool_fusion(aout_cr, mixv[0], mixv[1], mixv[2], mixv[4], qv,
                      mha_bv, mha_bo, fus_b, s2b, ap)

    attn_cm.__exit__(None, None, None)

    tail_cm = tc.tile_pool(name="tail", bufs=1)
    tp = tail_cm.__enter__()
    p.sbp = tp

    conv_w = p.chunk(tp, "conv_w", 16384)
    c1T = p.views(conv_w, NT, width=2048)
    c2T = p.views(conv_w, NF, off=8192)
    A_lhsT = p.views(p.chunk(tp, "A_lhsT", 2048, dtype=F32), NT)
    wtT = p.views(p.chunk(tp, "trend_wT", 2048), NT)

    # fused = sa_attn @ Wc^T + (summary @ W2^T + fus_b'); y = x_sa + fused
    s2_rep = mktile(tp, [128, 512], F32, "s2_rep")
    nc.gpsimd.partition_broadcast(s2_rep[:], s2b[:])
    y_tiles = []
    for tc_ in range(NT):
        ps = p.ps_mm()
        for fc in range(NT):
            nc.tensor.matmul(
                ps[:], aout_sa[fc][:, 128 * tc_:128 * (tc_ + 1)],
                mixv[3][fc][:], start=(fc == 0), stop=(fc == NT - 1))
        y = mktile(tp, [128, 512], F32, "y", bufs=NT)
        nc.vector.tensor_tensor(y[:], ps[:], xsa_tok[tc_][:], op=ALU.add)
        nc.vector.tensor_tensor(y[:], y[:], s2_rep[:], op=ALU.add)
        y_tiles.append(y)

    # normf (token-major my_ln); beta cancels, gamma applied
    xln = [mktile(tp, [128, 512], F32, "xln", bufs=NT) for _ in range(NT)]
    xs = [mktile(tp, [128, 512], F32, "xs", bufs=NT) for _ in range(NT)]
    mu, rstd = p.my_ln_stats(y_tiles, "lnf")
    xh = []
    for c in range(NT):
        o = mktile(tp, [128, 512], F32, "ln_xh", bufs=4)
        nc.vector.tensor_scalar(r(o[:]), y_tiles[c][:], mu[:, c:c + 1],
                                rstd[:, c:c + 1], op0=ALU.subtract,
                                op1=ALU.mult)
        xh.append(o)
    p.seqmean_sub(xh, "lnf_sm", gmul=gf_rep, outs=xln)

    # decomp1: xs = xln - A @ xln
    t1_ps = p.ma_matmul(A_lhsT, xln)
    for c in range(NT):
        nc.vector.tensor_tensor(xs[c][:], xln[c][:], t1_ps[c][:],
                                op=ALU.subtract)

    # norm3 (gamma folded into conv1, beta cancels)
    mu3, rstd3 = p.my_ln_stats(xs, "ln3")
    xh3 = []
    for c in range(NT):
        o = mktile(tp, [128, 512], F32, "ln_xh", bufs=4)
        nc.vector.tensor_scalar(r(o[:]), xs[c][:], mu3[:, c:c + 1],
                                rstd3[:, c:c + 1], op0=ALU.subtract,
                                op1=ALU.mult)
        xh3.append(o)
    xn = p.seqmean_sub(xh3, "ln3_sm")

    # ffn: transpose -> conv1+relu (feature-major) -> conv2 (token-major)
    xnT = [mktile(tp, [128, 512], BF16, "xnT", bufs=NT) for _ in range(NT)]
    for rr in range(NT):
        for cc in range(NT):
            tps = p.ps_sc()
            nc.tensor.transpose(tps[:, 0:128],
                                xn[rr][:, 128 * cc:128 * (cc + 1)],
                                cols[:, COLS_W - 128:COLS_W])
            nc.any.tensor_copy(xnT[cc][:, 128 * rr:128 * (rr + 1)],
                               tps[:, 0:128])
    relu = []
    for m in range(NF):
        ps = p.ps_mm()
        for fc in range(NT):
            nc.tensor.matmul(ps[:], c1T[fc][:, 128 * m:128 * (m + 1)],
                             xnT[fc][:], start=(fc == 0), stop=(fc == NT - 1))
        o = mktile(tp, [128, 512], BF16, "relu", bufs=NF)
        nc.scalar.activation(o[:], ps[:], ACTF.Relu)
        relu.append(o)
    y2 = []
    for tc_ in range(NT):
        ps = p.ps_mm()
        for m in range(NF):
            nc.tensor.matmul(ps[:], relu[m][:, 128 * tc_:128 * (tc_ + 1)],
                             c2T[m][:], start=(m == 0), stop=(m == NF - 1))
        o = mktile(tp, [128, 512], F32, "y2", bufs=NT)
        nc.vector.tensor_tensor(r(o[:]), ps[:], xs[tc_][:], op=ALU.add)
        y2.append(o)

    # decomp2 + output x
    t2_ps = p.ma_matmul(A_lhsT, y2)
    for c in range(NT):
        o = mktile(tp, [128, 512], F32, "x2_out", bufs=2)
        nc.vector.tensor_tensor(o[:], y2[c][:], t2_ps[c][:], op=ALU.subtract)
        nc.sync.dma_start(Hd["out_x"][128 * c:128 * (c + 1), :], o[:])

    # trend = (A @ (xln + y2)) @ trend_w^T + trend_b
    z = []
    for c in range(NT):
        o = mktile(tp, [128, 512], F32, "z", bufs=NT)
        nc.vector.tensor_tensor(r(o[:]), xln[c][:], y2[c][:], op=ALU.add)
        z.append(o)
    azT = []
    for m in range(NT):
        ps = p.ps_mm()
        for tc_ in range(NT):
            nc.tensor.matmul(ps[:], r(z[tc_][:, 128 * m:128 * (m + 1)]),
                             r(A_lhsT[tc_][:]), start=(tc_ == 0),
                             stop=(tc_ == NT - 1))
        o = mktile(tp, [128, 512], BF16, "azT", bufs=NT)
        nc.any.tensor_copy(o[:], ps[:])
        azT.append(o)
    for tc_ in range(NT):
        ps = p.ps_mm()
        nc.tensor.matmul(ps[:], r(p.ones1x128[:]), r(trend_b[:]),
                         start=True, stop=False)
        for fc in range(NT):
            nc.tensor.matmul(ps[:], azT[fc][:, 128 * tc_:128 * (tc_ + 1)],
                             wtT[fc][:], start=False, stop=(fc == NT - 1))
        o = mktile(tp, [128, 512], F32, "tr_out", bufs=1)
        nc.any.tensor_copy(o[:], ps[:])
        nc.sync.dma_start(Hd["out_trend"][128 * tc_:128 * (tc_ + 1), :], o[:])

    tail_cm.__exit__(None, None, None)
    psp_cm.__exit__(None, None, None)
    keep_cm.__exit__(None, None, None)


def build_program(share_bias):
    nc = bacc.Bacc("TRN2", target_bir_lowering=False, debug=False)
    shared, per_core, outs = _specs(share_bias)
    Hd = {}
    for name, shape, dt in shared + per_core:
        Hd[name] = nc.dram_tensor(name, list(shape), dt, kind="ExternalInput")
    for name, shape, dt in outs:
        Hd[name] = nc.dram_tensor(name, list(shape), dt, kind="ExternalOutput")
    with tile.TileContext(nc) as tc:
        emit(tc, nc, Hd, share_bias)
    nc.compile()
    return nc


# ----------------------------------------------------------------------------
# entry point
# ----------------------------------------------------------------------------

_LAST_EXEC_NS = None


def _in_maps(inputs):
    sh, per_core, share_bias = host_prepare(inputs)
    in_maps = []
    for b in range(B):
        m = dict(sh)
        m.update(per_core[b])
        in_maps.append(m)
    return in_maps, share_bias


def kernel(**inputs):
    global _LAST_EXEC_NS
    in_maps, share_bias = _in_maps(inputs)
    nc = build_program(share_bias)
    trace = os.environ.get("KBENCH_TRACE", "0") == "1"
    res = run_bass_kernel_spmd(nc, in_maps, list(range(B)), trace=trace)
    _LAST_EXEC_NS = res.exec_time_ns
    x = np.stack([res.results[b]["out_x"] for b in range(B)], axis=0)
    trend = np.stack([res.results[b]["out_trend"] for b in range(B)], axis=0)
    return np.stack([x, trend], axis=0)


def measure_exec_ns(inputs, iters=1):
    """On-device NEFF execution time via NTFF profiling (axon hook)."""
    in_maps, share_bias = _in_maps(inputs)
    nc = build_program(share_bias)
    res = run_bass_kernel_spmd(nc, in_maps, list(range(B)), trace=True)
    return res.exec_time_ns


# revision 27
# speedup vs baseline: 1.0364x; 1.0364x over previous
"""Trainium2 Bass kernel for nn_Model_15418932592810 (Autoformer-style decoder layer).

Sharding: data-parallel over batch B=8 across the 8 NeuronCores (one batch
per core, no collectives). Within a core, activations are feature-major
through the matmul chains; attention is keys-major (scores^T) with softmax
denominators riding as a ones-column in V; the series-decomp / layernorm
tail runs token-major.

v2 notes:
- all attention operands bf16 (K<128 fp32r matmuls run at ~half rate on HW)
- softmax/LN reciprocals via exp(-ln x) on the activation engine, staying
  inside one act-function table (exp/ln/relu/square/copy) -- no DVE
  RECIPROCAL (8.7us per call on a 1-partition row), no table reloads
- sa decay bias applied as a post-exp multiply (exp(s+b)=exp(s)exp(b));
  when all heads share lambda (true for these inputs) one packed tile
  serves all heads
- sa out-proj folded into fusion W1, cr out-proj folded into the ctx-pool
  K/V projections (host-side weight products); ctx-pool K bias dropped
  (adds a per-head constant to logits -- softmax invariant)
- weights DMA'd in a few large chunks, issued in consumption order
- trend's (A@z)^T produced directly by matmul (z as lhsT) instead of PE
  transposes
- PE warm-up spins while the first weight chunks land (HAM stays at 8/8)
"""
import math
import os
import numpy as np
import ml_dtypes

import concourse.bass as bass
import concourse.mybir as mybir
import concourse.tile as tile
from concourse import bacc
from concourse.bass_utils import run_bass_kernel_spmd

F32 = mybir.dt.float32
F32R = mybir.dt.float32r
BF16 = mybir.dt.bfloat16
AX = mybir.AxisListType
ALU = mybir.AluOpType
ACTF = mybir.ActivationFunctionType

B, L, D, H, DH, DFF, KMA = 8, 512, 512, 8, 64, 2048, 25
NT = 4          # number of 128-row tiles in a 512 dim
NF = DFF // 128
EPS = 1e-5
BF = ml_dtypes.bfloat16

# row indices in the packed (NROWS, 512) f32 "rows" tensor
R_SA_BV, R_CF_BV, R_CR_BV, R_MHA_BV, R_MHA_BO, R_FUS_B, R_TREND_B = range(7)
NROWS = 7
COLS_W = 192      # 24 bias cols + pad + identity(128) at the tail


def r(x):
    return x.bitcast(F32R)


def mktile(pool, shape, dtype, tag, bufs=None):
    return pool.tile(shape, dtype, name=tag, tag=tag, bufs=bufs)


# ----------------------------------------------------------------------------
# host-side input preparation
# ----------------------------------------------------------------------------

def _softplus(x):
    return np.logaddexp(0.0, x.astype(np.float64))


def _ma_matrix():
    pad = (KMA - 1) // 2
    A = np.zeros((L, L), dtype=np.float64)
    for i in range(L):
        for m in range(i, i + KMA):
            j = min(max(m - pad, 0), L - 1)
            A[i, j] += 1.0 / KMA
    return A


def _colpack(x):
    n = np.asarray(x).shape[0]
    return np.asarray(x, np.float64).reshape(n // 128, 128).T


def _T(w):
    return np.asarray(w, dtype=np.float64).T


def _pack(a):
    # (R, N) with R=128*c -> (128, c*N): column block c holds rows [128c,128c+128)
    a = np.asarray(a)
    rr, n = a.shape
    c = rr // 128
    return np.ascontiguousarray(
        a.reshape(c, 128, n).transpose(1, 0, 2).reshape(128, c * n))


def _Tp(w):
    return _pack(_T(w))


def host_prepare(inputs):
    ins = {k: np.asarray(v, dtype=np.float64) for k, v in inputs.items()}
    sh = {}
    s = 1.0 / math.sqrt(DH)

    qkv_w = ins["sa_qkv_w"]
    qkv_b = ins["sa_qkv_b"]
    att = {
        "sa": (qkv_w[:D] * s, qkv_b[:D] * s, qkv_w[D:2 * D], qkv_b[D:2 * D],
               qkv_w[2 * D:], qkv_b[2 * D:]),
        "cf": (ins["cf_q_w"] * s, ins["cf_q_b"] * s, ins["cf_k_w"],
               ins["cf_k_b"], ins["cf_v_w"], ins["cf_v_b"]),
        "cr": (ins["cr_q_w"] * s, ins["cr_q_b"] * s, ins["cr_k_w"],
               ins["cr_k_b"], ins["cr_v_w"], ins["cr_v_b"]),
    }

    def bfc(*mats):
        return np.ascontiguousarray(
            np.concatenate([_Tp(m) for m in mats], axis=1)).astype(BF)

    i = np.arange(L, dtype=np.float64)
    rel = i[None, :] - i[:, None]                  # rel[q, k] = k - q
    lf = _softplus(ins["sa_lam_f"])[:, None, None]
    lb = _softplus(ins["sa_lam_b"])[:, None, None]
    decay = np.where(rel[None] < 0, -lb * np.abs(rel[None]),
                     np.where(rel[None] > 0, -lf * rel[None], 0.0))
    edecay = np.exp(decay)                          # (H, L, L), in [0, 1]
    share_bias = bool(np.all(np.abs(edecay - edecay[:1]) < 1e-12))

    wq, bq, wk, bk, wv, bv = att["sa"]
    if share_bias:
        sa_chunk = np.concatenate(
            [bfc(wq, wk, wv), _pack(edecay[0].T).astype(BF)], axis=1)
    else:
        sa_chunk = np.concatenate(
            [bfc(wq, wk, wv)] + [_pack(m.T).astype(BF) for m in edecay],
            axis=1)
    sh["sa_w"] = np.ascontiguousarray(sa_chunk)

    sh["cf_w"] = bfc(att["cf"][0], att["cf"][2], att["cf"][4], ins["cf_o_w"])
    sh["cr_w"] = bfc(att["cr"][0], att["cr"][2], att["cr"][4])

    for p, lw in [("cf", "cf_logw"), ("cr", "cr_logw")]:
        w = np.exp(ins[lw])[:, None]
        ang = 2.0 * math.pi * w * i[None, :]       # (H, L)
        cs = np.stack([np.cos(ang), np.sin(ang)], axis=1)  # (H, 2, L)
        cs2 = cs.transpose(1, 0, 2).reshape(2, H * L)
        # rows 0-1 serve even heads, rows 64-65 odd heads, so consecutive
        # cs-init matmuls alternate PE row-groups (LDWEIGHTS overlap)
        cst = np.zeros((66, H * L))
        cst[0:2] = cs2
        cst[64:66] = cs2
        sh[f"{p}_cs"] = np.ascontiguousarray(cst).astype(BF)

    # folded weights
    wqm = ins["mha_in_w"][:D]
    bqm = ins["mha_in_b"][:D]
    wkm = ins["mha_in_w"][D:2 * D]
    wvm = ins["mha_in_w"][2 * D:]
    bvm = ins["mha_in_b"][2 * D:]
    wo_cr, bo_cr = ins["cr_o_w"], ins["cr_o_b"]
    wk_f = wkm @ wo_cr
    wv_f = wvm @ wo_cr
    bv_f = bvm + wvm @ bo_cr
    w1 = ins["fusion_w"][:, :D]
    wc = w1 @ ins["sa_out_w"]
    fus_b = ins["fusion_b"] + w1 @ ins["sa_out_b"]
    sh["mix_w"] = bfc(wk_f, wv_f, ins["mha_out_w"], wc, ins["fusion_w"][:, D:])

    sh["conv_w"] = np.ascontiguousarray(np.concatenate(
        [_Tp(ins["conv1_w"] * ins["norm3_g"][None, :]),
         _Tp(ins["conv2_w"])], axis=1)).astype(BF)

    sh["trend_wT"] = _Tp(ins["trend_w"]).astype(BF)
    sh["A_lhsT"] = _Tp(_ma_matrix()).astype(np.float32)
    sh["gf_rep"] = np.ascontiguousarray(
        np.tile(np.asarray(ins["normf_g"], np.float32)[None, :], (128, 1)))

    cols = np.zeros((128, COLS_W), np.float64)
    for idx, pfx in enumerate(("sa", "cf", "cr")):
        cols[:, 8 * idx:8 * idx + 4] = _colpack(att[pfx][1])
        cols[:, 8 * idx + 4:8 * idx + 8] = _colpack(att[pfx][3])
    cols[:, 24:28] = _colpack(ins["cf_o_b"])
    cols[:, COLS_W - 128:] = np.eye(128)
    sh["cols"] = cols.astype(np.float32)

    rows = np.zeros((NROWS, 512), np.float64)
    rows[R_SA_BV] = att["sa"][5]
    rows[R_CF_BV] = att["cf"][5]
    rows[R_CR_BV] = att["cr"][5]
    rows[R_MHA_BV] = bv_f
    rows[R_MHA_BO] = ins["mha_out_b"]
    rows[R_FUS_B] = fus_b
    rows[R_TREND_B] = ins["trend_b"]
    sh["rows"] = rows.astype(np.float32)

    qvec = (ins["global_q"].reshape(D) @ wqm.T + bqm) * s
    qvp = np.zeros((128, 32), np.float64)
    for h in range(H):
        fc = (64 * h) // 128
        r0 = 64 * h - 128 * fc
        qvp[r0:r0 + 64, 8 * fc + h] = qvec[64 * h:64 * h + 64]
    sh["qv_bf"] = qvp.astype(BF)

    per_core = []
    for b in range(B):
        x_sa = np.asarray(inputs["x_sa"][b], np.float64)
        per_core.append({
            "xsaT": _Tp(x_sa).astype(BF),
            "xq": np.ascontiguousarray(np.concatenate(
                [_Tp(np.asarray(inputs["x_q1"][b], np.float64)),
                 _Tp(np.asarray(inputs["x_q2"][b], np.float64))],
                axis=1)).astype(BF),
            "xsa_tok": _pack(x_sa).astype(np.float32),
        })
    return sh, per_core, share_bias


# ----------------------------------------------------------------------------
# program builder
# ----------------------------------------------------------------------------

def _specs(share_bias):
    sa_w_cols = 3 * 2048 + (2048 if share_bias else 8 * 2048)
    shared = [
        ("cols", (128, COLS_W), F32), ("rows", (NROWS, 512), F32),
        ("qv_bf", (128, 32), BF16),
        ("sa_w", (128, sa_w_cols), BF16),
        ("cf_w", (128, 4 * 2048), BF16), ("cf_cs", (66, H * L), BF16),
        ("cr_w", (128, 3 * 2048), BF16), ("cr_cs", (66, H * L), BF16),
        ("mix_w", (128, 5 * 2048), BF16),
        ("conv_w", (128, 16384), BF16),
        ("trend_wT", (128, 2048), BF16),
        ("A_lhsT", (128, 2048), F32),
        ("gf_rep", (128, 512), F32),
    ]
    per_core = [
        ("xsaT", (128, 2048), BF16), ("xq", (128, 4096), BF16),
        ("xsa_tok", (128, 2048), F32),
    ]
    outs = [("out_x", (512, 512), F32), ("out_trend", (512, 512), F32)]
    return shared, per_core, outs


def hslice(tiles, h):
    t = tiles[h // 2]
    off = 64 * (h % 2)
    return t[off:off + 64, :]


class Prog:
    def __init__(self, nc, tc, Hd):
        self.nc, self.tc, self.Hd = nc, tc, Hd

    # ------------------------------------------------------------------
    def chunk(self, pool, name, width, dtype=BF16, tag=None):
        nc = self.nc
        t = mktile(pool, [128, width], dtype, tag or name)
        if dtype == F32:
            nc.sync.dma_start(r(t[:]), r(self.Hd[name][:]))
        else:
            nc.sync.dma_start(t[:], self.Hd[name][:])
        return t

    def rowtile(self, pool, ridx, tag):
        """(1, 512) f32 tile at partition 0, loaded from rows[ridx]."""
        t = mktile(pool, [1, 512], F32, tag)
        self.nc.sync.dma_start(r(t[:]), r(self.Hd["rows"][ridx:ridx + 1, :]))
        return t

    @staticmethod
    def views(t, n, width=512, off=0):
        return [t[:, off + width * c:off + width * (c + 1)] for c in range(n)]

    def ps_mm(self):
        return mktile(self.psp, [128, 512], F32, "mm", bufs=2)

    def ps_sc(self):
        return mktile(self.psp, [128, 512], F32, "sc", bufs=2)

    def ps_sc2(self):
        return mktile(self.psp, [128, 1024], F32, "sc", bufs=2)

    def ps_av(self):
        return mktile(self.psp, [65, 512], F32, "av", bufs=2)

    # ------------------------------------------------------------------
    def proj_fm(self, xT, wT, b_c, tag, dtype=BF16, bufs=5):
        """Feature-major projection: out^T = W @ x^T + b, NT tiles (128,512)."""
        nc = self.nc
        outs = []
        for m in range(NT):
            ps = self.ps_mm()
            for kc in range(NT):
                nc.tensor.matmul(ps[:], wT[kc][:, 128 * m:128 * (m + 1)],
                                 xT[kc][:], start=(kc == 0),
                                 stop=(kc == NT - 1))
            o = mktile(self.sbp, [128, 512], dtype, tag, bufs=bufs)
            nc.vector.tensor_scalar(o[:], ps[:], b_c[:, m:m + 1], None,
                                    op0=ALU.add)
            outs.append(o)
        return outs

    def proj_tok_aug(self, xT, wvT, bv_row, tag):
        """Token-major V projection, ones column interleaved per head (bf16)."""
        nc = self.nc
        bv_rep = mktile(self.sbp, [128, 512], F32, "at_bvrep", bufs=1)
        nc.gpsimd.partition_broadcast(bv_rep[:], bv_row[:])
        outs = []
        for kc in range(NT):
            ps = self.ps_mm()
            for fc in range(NT):
                nc.tensor.matmul(ps[:], xT[fc][:, 128 * kc:128 * (kc + 1)],
                                 wvT[fc][:], start=(fc == 0),
                                 stop=(fc == NT - 1))
            o = mktile(self.sbp, [128, 520], BF16, tag, bufs=5)
            nc.vector.memset(o[:], 1.0)
            ov = o[:].rearrange("p (h c) -> p h c", c=65)
            nc.vector.tensor_tensor(
                ov[:, :, 0:64], ps[:].rearrange("p (h c) -> p h c", c=64),
                bv_rep[:].rearrange("p (h c) -> p h c", c=64), op=ALU.add)
            outs.append(o)
        return outs

    # ------------------------------------------------------------------
    def attention(self, qinT, kvinT, wqT, wkT, wvT, bq_c, bk_c, bv_row,
                  decay=None, cs=None, out_proj=None, aout_pool=None,
                  aout_tag="at_ao"):
        """8-head attention; returns feature-major bf16 NT tiles: the
        pre-out-proj attention concat, or (with out_proj=(woT, bo_c, residT))
        the projected + residual output."""
        nc = self.nc
        qT = self.proj_fm(qinT, wqT, bq_c, "at_q")
        kT = self.proj_fm(kvinT, wkT, bk_c, "at_k")
        vaug = self.proj_tok_aug(kvinT, wvT, bv_row, "at_v")
        apool = aout_pool or self.sbp

        aout = []
        for half in range(2):
            expT = [mktile(self.sbp, [128, 2048], BF16, "at_exp", bufs=4)
                    for _ in range(NT)]
            avs_sb = []
            zcat = mktile(self.sbp, [1, 2048], F32, "at_zc", bufs=1)
            for pair in range(2):
                ha, hb = 4 * half + 2 * pair, 4 * half + 2 * pair + 1
                for kc in range(NT):
                    # two heads per 2-bank psum tile; emission order
                    # cs-a, cs-b, score-a, score-b alternates PE row-groups
                    # on every instruction so LDWEIGHTS overlaps matmuls
                    ps = self.ps_sc2()
                    for j, h in enumerate((ha, hb)):
                        out = ps[:, 512 * j:512 * (j + 1)]
                        if cs is not None:
                            cr0 = 0 if h % 2 == 0 else 64
                            nc.tensor.matmul(
                                out,
                                cs[cr0:cr0 + 2,
                                   512 * h + 128 * kc:512 * h + 128 * (kc + 1)],
                                cs[cr0:cr0 + 2, 512 * h:512 * (h + 1)],
                                start=True, stop=False)
                        nc.tensor.matmul(
                            out,
                            hslice(kT, h)[:, 128 * kc:128 * (kc + 1)],
                            hslice(qT, h), start=(cs is None), stop=True)
                    esl = expT[kc][:, 1024 * pair:1024 * (pair + 1)]
                    nc.scalar.activation(esl, ps[:], ACTF.Exp)
                    if decay is not None:
                        dsl = decay[ha] if isinstance(decay, list) else decay
                        db = dsl[:, 512 * kc:512 * (kc + 1)]
                        nc.vector.tensor_tensor(
                            esl[:, 0:512], esl[:, 0:512], db, op=ALU.mult)
                        if isinstance(decay, list):
                            db = decay[hb][:, 512 * kc:512 * (kc + 1)]
                        nc.vector.tensor_tensor(
                            esl[:, 512:1024], esl[:, 512:1024], db,
                            op=ALU.mult)
                for j, h in enumerate((ha, hb)):
                    av = self.ps_av()
                    for kc in range(NT):
                        nc.tensor.matmul(
                            av[:], vaug[kc][:, 65 * h:65 * h + 65],
                            expT[kc][:, 1024 * pair + 512 * j:
                                     1024 * pair + 512 * (j + 1)],
                            start=(kc == 0), stop=(kc == NT - 1))
                    h4 = 2 * pair + j
                    nc.vector.tensor_copy(
                        zcat[0:1, 512 * h4:512 * (h4 + 1)], av[64:65, :])
                    asb = mktile(self.sbp, [64, 512], BF16, "at_asb", bufs=6)
                    nc.vector.tensor_copy(asb[:], av[0:64, :])
                    avs_sb.append(asb)
            # softmax epilogue with a single Ln per half: gather the four
            # Z-rows into one (1,2048) row, one Ln, one wide exp(-x) -- the
            # rcp shares the exp table with the score exps, so each half
            # costs at most 2 act-table loads
            nc.scalar.activation(zcat[:], zcat[:], ACTF.Ln)
            rcp = mktile(self.sbp, [1, 2048], BF16, "at_rcp", bufs=1)
            nc.scalar.activation(rcp[:], zcat[:], ACTF.Exp, scale=-1.0)
            for h4 in range(4):
                h = 4 * half + h4
                rep = mktile(self.sbp, [64, 512], BF16, "at_rep", bufs=2)
                nc.gpsimd.partition_broadcast(
                    rep[:], rcp[0:1, 512 * h4:512 * (h4 + 1)])
                if h % 2 == 0:
                    pair = mktile(apool, [128, 512], BF16, aout_tag, bufs=4)
                    aout.append(pair)
                off = 64 * (h % 2)
                nc.vector.tensor_tensor(aout[h // 2][off:off + 64, :],
                                        avs_sb[h4][:], rep[:], op=ALU.mult)
        if out_proj is None:
            return aout
        woT, bo_c, residT = out_proj
        outs = []
        for m in range(NT):
            ps = self.ps_mm()
            for c in range(NT):
                nc.tensor.matmul(ps[:], woT[c][:, 128 * m:128 * (m + 1)],
                                 aout[c][:], start=(c == 0),
                                 stop=(c == NT - 1))
            o = mktile(self.sbp, [128, 512], BF16, "at_enr", bufs=4)
            nc.vector.tensor_scalar(o[:], ps[:], bo_c[:, m:m + 1], None,
                                    op0=ALU.add)
            nc.vector.tensor_tensor(o[:], o[:], residT[m][:], op=ALU.add)
            outs.append(o)
        return outs

    # ------------------------------------------------------------------
    def ctx_pool_fusion(self, caT, wkT, wvT, woT, w2T, qv, bv_row, bo_row,
                        fus_b_row, s2b, pool):
        """1-query MultiheadAttention pooling feeding the fusion summary row:
        s2b = Wf2 @ summary + fus_b'. K-proj bias dropped (softmax-invariant).
        Tiles live in `pool` (keep) so the attn pool can close beforehand and
        the tail overlaps the ctx chain."""
        nc = self.nc
        sav_sbp, self.sbp = self.sbp, pool
        kT = self.proj_fm(caT, wkT, self.zero_col[:, 0:4], "mha_k", bufs=4)
        v_tok = []
        for kc in range(NT):
            ps = self.ps_mm()
            nc.tensor.matmul(ps[:], r(self.ones1x128[:]), r(bv_row[:]),
                             start=True, stop=False)
            for fc in range(NT):
                nc.tensor.matmul(ps[:], caT[fc][:, 128 * kc:128 * (kc + 1)],
                                 wvT[fc][:], start=False, stop=(fc == NT - 1))
            o = mktile(self.sbp, [128, 512], BF16, "mha_v", bufs=4)
            nc.any.tensor_copy(o[:], ps[:])
            v_tok.append(o)
        exps = []
        for kc in range(NT):
            ps = self.ps_sc()
            for fc in range(NT):
                nc.tensor.matmul(ps[:, 0:8],
                                 kT[fc][:, 128 * kc:128 * (kc + 1)],
                                 qv[:, 8 * fc:8 * (fc + 1)],
                                 start=(fc == 0), stop=(fc == NT - 1))
            e = mktile(self.sbp, [128, 8], BF16, "mha_exp", bufs=4)
            nc.scalar.activation(e[:], ps[:, 0:8], ACTF.Exp)
            exps.append(e)
        sps = self.ps_sc()
        for kc in range(NT):
            nc.tensor.matmul(sps[0:1, 0:8], self.ones_col_bf[:, 0:1],
                             exps[kc][:], start=(kc == 0), stop=(kc == NT - 1))
        lnz = mktile(self.sbp, [1, 8], F32, "mha_lnz")
        nc.scalar.activation(lnz[:], sps[0:1, 0:8], ACTF.Ln)
        rsum = mktile(self.sbp, [1, 8], F32, "mha_rsum")
        nc.scalar.activation(rsum[:], lnz[:], ACTF.Exp, scale=-1.0)
        rrep = mktile(self.sbp, [128, 8], F32, "mha_rrep")
        nc.gpsimd.partition_broadcast(rrep[:], rsum[:])
        for kc in range(NT):
            nc.vector.tensor_tensor(exps[kc][:], exps[kc][:], rrep[:],
                                    op=ALU.mult)

        yps = self.ps_av()
        for h in range(H):
            for kc in range(NT):
                nc.tensor.matmul(yps[0:64, 2 * h:2 * h + 1],
                                 v_tok[kc][:, 64 * h:64 * h + 64],
                                 exps[kc][:, h:h + 1],
                                 start=(kc == 0), stop=(kc == NT - 1))
        y_sb = mktile(self.sbp, [128, NT], BF16, "mha_y")
        nc.vector.memset(y_sb[:], 0.0)
        for h in range(H):
            off = 64 * (h % 2)
            nc.vector.tensor_copy(y_sb[off:off + 64, h // 2:h // 2 + 1],
                                  yps[0:64, 2 * h:2 * h + 1])

        summ = mktile(self.sbp, [128, 8], BF16, "mha_summ")
        sps2 = self.ps_mm()
        for mc in range(NT):
            nc.tensor.matmul(sps2[:, 2 * mc:2 * mc + 1],
                             bo_row[0:1, 128 * mc:128 * (mc + 1)],
                             self.ones1x1[:], start=True, stop=False)
            for c in range(NT):
                nc.tensor.matmul(sps2[:, 2 * mc:2 * mc + 1],
                                 woT[c][:, 128 * mc:128 * (mc + 1)],
                                 y_sb[:, c:c + 1], start=False,
                                 stop=(c == NT - 1))
        for mc in range(NT):
            nc.vector.tensor_copy(summ[:, 2 * mc:2 * mc + 1],
                                  sps2[:, 2 * mc:2 * mc + 1])

        ps = self.ps_sc()
        nc.tensor.matmul(ps[0:1, :], r(self.ones1x1[:]), r(fus_b_row[:]),
                         start=True, stop=False)
        for fc in range(NT):
            nc.tensor.matmul(ps[0:1, :], summ[:, 2 * fc:2 * fc + 1],
                             w2T[fc][:], start=False, stop=(fc == NT - 1))
        nc.vector.tensor_copy(r(s2b[:]), ps[0:1, :])
        self.sbp = sav_sbp

    # ------------------------------------------------------------------
    def my_ln_stats(self, y_tiles, tag):
        nc = self.nc
        sums = mktile(self.sbp, [128, NT], F32, f"{tag}_sums")
        sumsq = mktile(self.sbp, [128, NT], F32, f"{tag}_sumsq")
        for c in range(NT):
            nc.vector.tensor_reduce(sums[:, c:c + 1], y_tiles[c][:], axis=AX.X,
                                    op=ALU.add)
            sq = mktile(self.sbp, [128, 512], BF16, "lnsq", bufs=2)
            nc.scalar.activation(sq[:], y_tiles[c][:], ACTF.Square,
                                 accum_out=sumsq[:, c:c + 1])
        mu = mktile(self.sbp, [128, NT], F32, f"{tag}_mu")
        nc.vector.tensor_scalar(mu[:], sums[:], 1.0 / D, None, op0=ALU.mult)
        var = mktile(self.sbp, [128, NT], F32, f"{tag}_var")
        nc.vector.tensor_tensor(var[:], mu[:], mu[:], op=ALU.mult)
        msq = mktile(self.sbp, [128, NT], F32, f"{tag}_msq")
        nc.vector.tensor_scalar(msq[:], sumsq[:], 1.0 / D, None, op0=ALU.mult)
        nc.vector.tensor_tensor(var[:], msq[:], var[:], op=ALU.subtract)
        # rstd = exp(-0.5 * ln(var + eps)): stays in the exp/ln act table
        lnv = mktile(self.sbp, [128, NT], F32, f"{tag}_lnv")
        nc.scalar.activation(lnv[:], var[:], ACTF.Ln,
                             bias=self.eps_col[:, 0:1])
        rstd = mktile(self.sbp, [128, NT], F32, f"{tag}_rstd")
        nc.scalar.activation(rstd[:], lnv[:], ACTF.Exp, scale=-0.5)
        return mu, rstd

    def seqmean_sub(self, xh_tiles, tag, gmul=None, outs=None):
        nc = self.nc
        ps = self.ps_sc()
        for c in range(NT):
            nc.tensor.matmul(ps[0:1, :], r(self.ones_col[:, 0:1]),
                             r(xh_tiles[c][:]), start=(c == 0),
                             stop=(c == NT - 1))
        row = mktile(self.sbp, [1, 512], F32, "sm_row", bufs=1)
        nc.scalar.mul(row[:], ps[0:1, :], 1.0 / L)
        rep = mktile(self.sbp, [128, 512], F32, "sm_rep", bufs=1)
        nc.gpsimd.partition_broadcast(rep[:], row[:])
        if outs is None:
            outs = [mktile(self.sbp, [128, 512], F32, f"{tag}_out", bufs=4)
                    for _ in range(NT)]
        for c in range(NT):
            nc.vector.tensor_tensor(r(outs[c][:]), xh_tiles[c][:], rep[:],
                                    op=ALU.subtract)
            if gmul is not None:
                nc.vector.tensor_tensor(r(outs[c][:]), outs[c][:], gmul[:],
                                        op=ALU.mult)
        return outs

    def ma_matmul(self, A_lhsT, x_tiles):
        nc = self.nc
        pss = []
        for tc_ in range(NT):
            ps = self.ps_mm()
            for kc in range(NT):
                nc.tensor.matmul(ps[:],
                                 r(A_lhsT[kc][:, 128 * tc_:128 * (tc_ + 1)]),
                                 r(x_tiles[kc][:]), start=(kc == 0),
                                 stop=(kc == NT - 1))
            pss.append(ps)
        return pss


def emit(tc, nc, Hd, share_bias):
    p = Prog(nc, tc, Hd)
    keep_cm = tc.tile_pool(name="keep", bufs=1)
    keep = keep_cm.__enter__()
    psp_cm = tc.tile_pool(name="ps", bufs=1, space="PSUM")
    p.psp = psp_cm.__enter__()

    # constants: one cols DMA + memsets; per-row tiles from the rows tensor
    cols = p.chunk(keep, "cols", COLS_W, dtype=F32)
    p.identf = cols[:, COLS_W - 128:COLS_W]
    bcol = {}
    for idx, pfx in enumerate(("sa", "cf", "cr")):
        bcol[f"{pfx}_q"] = cols[:, 8 * idx:8 * idx + 4]
        bcol[f"{pfx}_k"] = cols[:, 8 * idx + 4:8 * idx + 8]
    cf_bo_c = cols[:, 24:28]
    qv = mktile(keep, [128, 32], BF16, "qv_bf")
    nc.sync.dma_start(qv[:], Hd["qv_bf"][:])
    p.zero_col = mktile(keep, [128, 4], F32, "zero_col")
    nc.vector.memset(p.zero_col[:], 0.0)
    p.ones_col = mktile(keep, [128, 1], F32, "ones_col")
    nc.vector.memset(p.ones_col[:], 1.0)
    p.ones_col_bf = mktile(keep, [128, 1], BF16, "ones_col_bf")
    nc.vector.memset(p.ones_col_bf[:], 1.0)
    p.ones1x128 = mktile(keep, [1, 128], F32, "ones1x128")
    nc.vector.memset(p.ones1x128[:], 1.0)
    p.ones1x1 = p.ones1x128[0:1, 0:1]
    p.eps_col = mktile(keep, [128, 1], F32, "eps_col")
    nc.vector.memset(p.eps_col[:], EPS)
    sa_bv = p.rowtile(keep, R_SA_BV, "sa_bv")
    cf_bv = p.rowtile(keep, R_CF_BV, "cf_bv")
    cr_bv = p.rowtile(keep, R_CR_BV, "cr_bv")
    mha_bv = p.rowtile(keep, R_MHA_BV, "mha_bv")
    mha_bo = p.rowtile(keep, R_MHA_BO, "mha_bo")
    fus_b = p.rowtile(keep, R_FUS_B, "fus_b")
    trend_b = p.rowtile(keep, R_TREND_B, "trend_b")
    s2b = mktile(keep, [1, 512], F32, "s2b_row")

    # PE warm-up spins on the identity while the first weight chunks land
    for _ in range(24):
        ps = p.ps_sc()
        nc.tensor.matmul(ps[:, 0:128], r(p.identf), r(p.identf),
                         start=True, stop=True)

    attn_cm = tc.tile_pool(name="attn", bufs=1)
    ap = attn_cm.__enter__()
    p.sbp = ap

    # weight chunks, issued in consumption order
    xsaT = p.views(p.chunk(ap, "xsaT", 2048), NT)
    sa_w_cols = 3 * 2048 + (2048 if share_bias else 8 * 2048)
    sa_w = p.chunk(ap, "sa_w", sa_w_cols)
    xq_t = p.chunk(ap, "xq", 4096)
    xq1T, xq2T = p.views(xq_t, NT), p.views(xq_t, NT, off=2048)
    cf_w = p.chunk(ap, "cf_w", 4 * 2048)
    cf_cs = mktile(ap, [66, H * L], BF16, "cf_cs")
    nc.sync.dma_start(cf_cs[:], Hd["cf_cs"][:])
    cr_w = p.chunk(ap, "cr_w", 3 * 2048)
    cr_cs = mktile(ap, [66, H * L], BF16, "cr_cs")
    nc.sync.dma_start(cr_cs[:], Hd["cr_cs"][:])
    mix_w = p.chunk(keep, "mix_w", 5 * 2048)
    xsa_tok = p.views(p.chunk(keep, "xsa_tok", 2048, dtype=F32), NT)
    gf_rep = p.chunk(keep, "gf_rep", 512, dtype=F32)

    sa_wq = p.views(sa_w, NT)
    sa_wk = p.views(sa_w, NT, off=2048)
    sa_wv = p.views(sa_w, NT, off=2 * 2048)
    if share_bias:
        sa_decay = sa_w[:, 3 * 2048:4 * 2048]
    else:
        sa_decay = [sa_w[:, (3 + h) * 2048:(4 + h) * 2048] for h in range(H)]

    aout_sa = p.attention(xsaT, xsaT, sa_wq, sa_wk, sa_wv,
                          bcol["sa_q"], bcol["sa_k"], sa_bv,
                          decay=sa_decay, aout_pool=keep, aout_tag="aout_sa")
    cfv = [p.views(cf_w, NT, off=2048 * i) for i in range(4)]
    enriched = p.attention(xq2T, xq1T, cfv[0], cfv[1], cfv[2],
                           bcol["cf_q"], bcol["cf_k"], cf_bv, cs=cf_cs,
                           out_proj=(cfv[3], cf_bo_c, xq2T))
    crv = [p.views(cr_w, NT, off=2048 * i) for i in range(3)]
    aout_cr = p.attention(xq1T, enriched, crv[0], crv[1], crv[2],
                          bcol["cr_q"], bcol["cr_k"], cr_bv, cs=cr_cs,
                          aout_pool=keep, aout_tag="aout_cr")

    mixv = [p.views(mix_w, NT, off=2048 * i) for i in range(5)]
    p.ctx_pool_fusion(aout_cr, mixv[0], mixv[1], mixv[2], mixv[4], qv,
                      mha_bv, mha_bo, fus_b, s2b, keep)

    attn_cm.__exit__(None, None, None)

    tail_cm = tc.tile_pool(name="tail", bufs=1)
    tp = tail_cm.__enter__()
    p.sbp = tp

    conv_w = p.chunk(tp, "conv_w", 16384)
    c1T = p.views(conv_w, NT, width=2048)
    c2T = p.views(conv_w, NF, off=8192)
    A_lhsT = p.views(p.chunk(tp, "A_lhsT", 2048, dtype=F32), NT)
    wtT = p.views(p.chunk(tp, "trend_wT", 2048), NT)

    # fused = sa_attn @ Wc^T + (summary @ W2^T + fus_b'); y = x_sa + fused
    s2_rep = mktile(tp, [128, 512], F32, "s2_rep")
    nc.gpsimd.partition_broadcast(s2_rep[:], s2b[:])
    y_tiles = []
    for tc_ in range(NT):
        ps = p.ps_mm()
        for fc in range(NT):
            nc.tensor.matmul(
                ps[:], aout_sa[fc][:, 128 * tc_:128 * (tc_ + 1)],
                mixv[3][fc][:], start=(fc == 0), stop=(fc == NT - 1))
        y = mktile(tp, [128, 512], F32, "y", bufs=NT)
        nc.vector.tensor_tensor(y[:], ps[:], xsa_tok[tc_][:], op=ALU.add)
        nc.vector.tensor_tensor(y[:], y[:], s2_rep[:], op=ALU.add)
        y_tiles.append(y)

    # normf (token-major my_ln); beta cancels, gamma applied
    xln = [mktile(tp, [128, 512], F32, "xln", bufs=NT) for _ in range(NT)]
    xs = [mktile(tp, [128, 512], F32, "xs", bufs=NT) for _ in range(NT)]
    mu, rstd = p.my_ln_stats(y_tiles, "lnf")
    xh = []
    for c in range(NT):
        o = mktile(tp, [128, 512], F32, "ln_xh", bufs=4)
        nc.vector.tensor_scalar(r(o[:]), y_tiles[c][:], mu[:, c:c + 1],
                                rstd[:, c:c + 1], op0=ALU.subtract,
                                op1=ALU.mult)
        xh.append(o)
    p.seqmean_sub(xh, "lnf_sm", gmul=gf_rep, outs=xln)

    # decomp1: xs = xln - A @ xln
    t1_ps = p.ma_matmul(A_lhsT, xln)
    for c in range(NT):
        nc.vector.tensor_tensor(xs[c][:], xln[c][:], t1_ps[c][:],
                                op=ALU.subtract)

    # norm3 (gamma folded into conv1, beta cancels)
    mu3, rstd3 = p.my_ln_stats(xs, "ln3")
    xh3 = []
    for c in range(NT):
        o = mktile(tp, [128, 512], F32, "ln_xh", bufs=4)
        nc.vector.tensor_scalar(r(o[:]), xs[c][:], mu3[:, c:c + 1],
                                rstd3[:, c:c + 1], op0=ALU.subtract,
                                op1=ALU.mult)
        xh3.append(o)
    xn = p.seqmean_sub(xh3, "ln3_sm")

    # ffn: transpose -> conv1+relu (feature-major) -> conv2 (token-major)
    xnT = [mktile(tp, [128, 512], BF16, "xnT", bufs=NT) for _ in range(NT)]
    for rr in range(NT):
        for cc in range(NT):
            tps = p.ps_sc()
            nc.tensor.transpose(tps[:, 0:128],
                                xn[rr][:, 128 * cc:128 * (cc + 1)],
                                cols[:, COLS_W - 128:COLS_W])
            nc.any.tensor_copy(xnT[cc][:, 128 * rr:128 * (rr + 1)],
                               tps[:, 0:128])
    relu = []
    for m in range(NF):
        ps = p.ps_mm()
        for fc in range(NT):
            nc.tensor.matmul(ps[:], c1T[fc][:, 128 * m:128 * (m + 1)],
                             xnT[fc][:], start=(fc == 0), stop=(fc == NT - 1))
        o = mktile(tp, [128, 512], BF16, "relu", bufs=NF)
        nc.scalar.activation(o[:], ps[:], ACTF.Relu)
        relu.append(o)
    y2 = []
    for tc_ in range(NT):
        ps = p.ps_mm()
        for m in range(NF):
            nc.tensor.matmul(ps[:], relu[m][:, 128 * tc_:128 * (tc_ + 1)],
                             c2T[m][:], start=(m == 0), stop=(m == NF - 1))
        o = mktile(tp, [128, 512], F32, "y2", bufs=NT)
        nc.vector.tensor_tensor(r(o[:]), ps[:], xs[tc_][:], op=ALU.add)
        y2.append(o)

    # decomp2 + output x
    t2_ps = p.ma_matmul(A_lhsT, y2)
    for c in range(NT):
        o = mktile(tp, [128, 512], F32, "x2_out", bufs=2)
        nc.vector.tensor_tensor(o[:], y2[c][:], t2_ps[c][:], op=ALU.subtract)
        nc.sync.dma_start(Hd["out_x"][128 * c:128 * (c + 1), :], o[:])

    # trend = (A @ (xln + y2)) @ trend_w^T + trend_b
    z = []
    for c in range(NT):
        o = mktile(tp, [128, 512], F32, "z", bufs=NT)
        nc.vector.tensor_tensor(r(o[:]), xln[c][:], y2[c][:], op=ALU.add)
        z.append(o)
    azT = []
    for m in range(NT):
        ps = p.ps_mm()
        for tc_ in range(NT):
            nc.tensor.matmul(ps[:], r(z[tc_][:, 128 * m:128 * (m + 1)]),
                             r(A_lhsT[tc_][:]), start=(tc_ == 0),
                             stop=(tc_ == NT - 1))
        o = mktile(tp, [128, 512], BF16, "azT", bufs=NT)
        nc.any.tensor_copy(o[:], ps[:])
        azT.append(o)
    for tc_ in range(NT):
        ps = p.ps_mm()
        nc.tensor.matmul(ps[:], r(p.ones1x128[:]), r(trend_b[:]),
                         start=True, stop=False)
        for fc in range(NT):
            nc.tensor.matmul(ps[:], azT[fc][:, 128 * tc_:128 * (tc_ + 1)],
                             wtT[fc][:], start=False, stop=(fc == NT - 1))
        o = mktile(tp, [128, 512], F32, "tr_out", bufs=1)
        nc.any.tensor_copy(o[:], ps[:])
        nc.sync.dma_start(Hd["out_trend"][128 * tc_:128 * (tc_ + 1), :], o[:])

    tail_cm.__exit__(None, None, None)
    psp_cm.__exit__(None, None, None)
    keep_cm.__exit__(None, None, None)


def build_program(share_bias):
    nc = bacc.Bacc("TRN2", target_bir_lowering=False, debug=False)
    shared, per_core, outs = _specs(share_bias)
    Hd = {}
    for name, shape, dt in shared + per_core:
        Hd[name] = nc.dram_tensor(name, list(shape), dt, kind="ExternalInput")
    for name, shape, dt in outs:
        Hd[name] = nc.dram_tensor(name, list(shape), dt, kind="ExternalOutput")
    with tile.TileContext(nc) as tc:
        emit(tc, nc, Hd, share_bias)
    nc.compile()
    return nc


# ----------------------------------------------------------------------------
# entry point
# ----------------------------------------------------------------------------

_LAST_EXEC_NS = None


def _in_maps(inputs):
    sh, per_core, share_bias = host_prepare(inputs)
    in_maps = []
    for b in range(B):
        m = dict(sh)
        m.update(per_core[b])
        in_maps.append(m)
    return in_maps, share_bias


def kernel(**inputs):
    global _LAST_EXEC_NS
    in_maps, share_bias = _in_maps(inputs)
    nc = build_program(share_bias)
    trace = os.environ.get("KBENCH_TRACE", "0") == "1"
    res = run_bass_kernel_spmd(nc, in_maps, list(range(B)), trace=trace)
    _LAST_EXEC_NS = res.exec_time_ns
    x = np.stack([res.results[b]["out_x"] for b in range(B)], axis=0)
    trend = np.stack([res.results[b]["out_trend"] for b in range(B)], axis=0)
    return np.stack([x, trend], axis=0)


def measure_exec_ns(inputs, iters=1):
    """On-device NEFF execution time via NTFF profiling (axon hook)."""
    in_maps, share_bias = _in_maps(inputs)
    nc = build_program(share_bias)
    res = run_bass_kernel_spmd(nc, in_maps, list(range(B)), trace=True)
    return res.exec_time_ns
